# revision 2
# baseline (speedup 1.0000x reference)
"""Trainium2 Bass kernel for the DVR-JANET recurrent cell — lockstep variant.

Data-parallel over batch across 8 cores (8 sequences each), weights
replicated.  Unlike the staggered baseline (2 half-batches of 4 pipelined
half a step apart), ALL 8 sequences advance in lockstep: every
elementwise/activation op covers the full core batch (32-col tiles) and
every HxH weight-tile matmul streams 8 columns.  This halves the PE
weight-load count (the dominant PE cost: one fp16 FWL 128x128 load is
~26.7ns regardless of moving width) and halves the ACT/DVE instruction
count, whose ~60-190ns fixed instruction costs dominate at these widths.
The per-step serial chain (pt -> theta matmul -> sin -> casa -> bottom
matmul -> tanh -> update) is the wall-clock floor; all off-chain work
(gate-top matmuls, x-terms, biases, anew shift) is scheduled to hide
under it.  The whole update path runs in f16 so the DVE 2x mode applies.
sin/cos are evaluated in one ACT op over duplicated theta (theta'+pi/2
folded via the x-term matmul); sigmoid is rewritten as tanh; all biases
and rank-1 input terms are folded into two small stationary matmuls.
Final I/Q projections run as batched matmuls over the f16 state history
kept resident in SBUF.
"""

import functools
import numpy as np

import concourse.bacc as bacc
import concourse.mybir as mybir
from concourse import tile
import concourse.hw_specs as hw_specs
from concourse.bass_utils import run_bass_kernel_spmd

F32 = mybir.dt.float32
F16 = mybir.dt.float16
AF = mybir.ActivationFunctionType
OP = mybir.AluOpType

B, T, H = 64, 1024, 256
NCORES = 8
BL = B // NCORES          # batch per core = 8
CH = 128                  # XB chunk length (steps)
NT = 32                   # full weight tiles
PW = 64                   # projection window (8*PW f32 <= one PSUM bank)

AN_ENG = "v"              # engine for anew shift (v=vector g=gpsimd)
UPD_ENG = "v"             # engine for update chain d/m2/ns
PT_ENG = "v"              # engine for p = hI*hQ

# ---------------------------------------------------------------------------
# Pin the ACT table set to silu_and_others (contains sin AND tanh) so the
# compiler never inserts per-step table swaps.
_orig_tables = hw_specs.get_activation_tables.__wrapped__


def _pinned_tables(arch):
    full = _orig_tables(arch)
    return {name: (funcs if name == "silu_and_others" else set())
            for name, funcs in full.items()}


def _pin_tables():
    fn = functools.cache(_pinned_tables)
    hw_specs.get_activation_tables = fn
    if hasattr(bacc, "get_activation_tables"):
        bacc.get_activation_tables = fn


# ---------------------------------------------------------------------------
_PROG_CACHE = {}


def build_program(Tn=T, sb=0.0, data_T=None):
    """Build the 8-core SPMD program.  data_T sizes declared DRAM I/O so
    short-loop timing variants can share input maps with the full build."""
    if data_T is None:
        data_T = Tn
    key = (Tn, float(sb), data_T, AN_ENG, UPD_ENG, PT_ENG)
    if key in _PROG_CACHE:
        return _PROG_CACHE[key]
    _pin_tables()
    nch = max(1, (data_T + CH - 1) // CH)
    nc = bacc.Bacc("TRN2", target_bir_lowering=False, debug=False,
                   num_devices=NCORES)

    w1_d = nc.dram_tensor("W1", [128, NT * 128], F16, kind="ExternalInput").ap()
    xwa_d = nc.dram_tensor("XWA", [5, 128], F16, kind="ExternalInput").ap()
    xwg_d = nc.dram_tensor("XWG", [6, 128], F16, kind="ExternalInput").ap()
    onesg_d = nc.dram_tensor("ONESG", [6, 48], F16, kind="ExternalInput").ap()
    xba_d = nc.dram_tensor("XBA", [nch, 5, CH * 64], F16, kind="ExternalInput").ap()
    wp_d = nc.dram_tensor("WP", [128, 4], F16, kind="ExternalInput").ap()
    s0_d = nc.dram_tensor("S0", [128, 32], F16, kind="ExternalInput").ap()
    out_d = nc.dram_tensor("OUT", [1, 2 * data_T * BL], F32,
                           kind="ExternalOutput").ap()

    with tile.TileContext(nc, trace_sim=False) as tc:
        with (
            tc.tile_pool(name="const", bufs=1) as cpool,
            tc.tile_pool(name="buf", bufs=1) as bufpool,
            tc.tile_pool(name="xba", bufs=2) as xbapool,
            tc.tile_pool(name="work", bufs=3) as wpool,
            tc.tile_pool(name="pab", bufs=2, space="PSUM") as ppab,
            tc.tile_pool(name="pg", bufs=2, space="PSUM") as ppg,
            tc.tile_pool(name="proj", bufs=2, space="PSUM") as pproj,
        ):
            wt1 = cpool.tile([128, NT * 128], F16, tag="wt1")
            xwa = cpool.tile([5, 128], F16, tag="xwa")
            xwg = cpool.tile([6, 128], F16, tag="xwg")
            onesg = cpool.tile([6, 48], F16, tag="onesg")
            wp = cpool.tile([128, 4], F16, tag="wp")
            # slot t: state entering step t; 32 cols
            # [hI-j0(8) hI-j1(8) hQ-j0(8) hQ-j1(8)]; +2 slots for h0 and the
            # projection's strided-window padding.
            buf = bufpool.tile([128, 32 * (Tn + 2)], F16, tag="buf")

            nc.sync.dma_start(wt1[:], w1_d)
            nc.sync.dma_start(xwa[:], xwa_d)
            nc.sync.dma_start(xwg[:], xwg_d)
            nc.sync.dma_start(onesg[:], onesg_d)
            nc.sync.dma_start(wp[:], wp_d)
            nc.sync.dma_start(buf[:, 0:32], s0_d)

            def wtile(i):
                return wt1[:, 128 * i:128 * (i + 1)]

            def bcast2(ap, w):
                return ap.rearrange("p (o f) -> p o f", o=1).broadcast_to([128, 2, w])

            chunk = {"xba": None}
            ane = {"v": nc.vector, "g": nc.gpsimd}[AN_ENG]
            ue = {"v": nc.vector, "g": nc.gpsimd}[UPD_ENG]
            pte = {"v": nc.vector, "g": nc.gpsimd}[PT_ENG]

            for t in range(Tn):
                s = t % CH
                if s == 0:
                    cc = t // CH
                    chunk["xba"] = xbapool.tile([5, CH * 64], F16, tag="xba",
                                                name="xbat")
                    nc.sync.dma_start(chunk["xba"][:], xba_d[cc])
                slot = buf[:, 32 * t:32 * t + 32]
                nslot = buf[:, 32 * (t + 1):32 * (t + 1) + 32]
                # pab cols: [th-j0(8) th'-j0(8) th-j1(8) th'-j1(8) |
                #            a-j0 a-j0 a-j1 a-j1] (a written twice via
                # broadcast rhs so the casa multiplier needs no bcast AP)
                pab = ppab.tile([128, 64], F32, tag="pab")
                pg = ppg.tile([128, 48], F32, tag="pg")
                pt = wpool.tile([128, 16], F16, tag="pt")
                sc = wpool.tile([128, 32], F16, tag="sc")
                casa = wpool.tile([128, 32], F16, tag="casa")
                d32 = wpool.tile([128, 32], F16, tag="d32")
                m232 = wpool.tile([128, 32], F16, tag="m232")

                # ---- s1: gate bank (off-chain parts first) ---------------
                nc.tensor.matmul(pg[:], xwg[:], onesg[:], start=True,
                                 stop=False)
                for j in (0, 1):
                    for k in (0, 1):
                        # gc-top: streams hI
                        nc.tensor.matmul(pg[:, 8 * j:8 * j + 8],
                                         wtile(16 + 2 * j + k),
                                         slot[:, 8 * k:8 * k + 8],
                                         start=False, stop=False)
                        # gs-top: streams hQ
                        nc.tensor.matmul(pg[:, 16 + 8 * j:16 + 8 * j + 8],
                                         wtile(20 + 2 * j + k),
                                         slot[:, 16 + 8 * k:16 + 8 * k + 8],
                                         start=False, stop=False)
                # p = hI*hQ  (chain head)
                pte.tensor_mul(pt[:], slot[:, 0:16], slot[:, 16:32])
                # ---- theta/theta'/a bank --------------------------------
                nc.tensor.matmul(pab[:], xwa[:],
                                 chunk["xba"][:, 64 * s:64 * s + 64],
                                 start=True, stop=False)
                def ptb(k):
                    return pt[:, 8 * k:8 * k + 8] \
                        .rearrange("p (o f) -> p o f", o=1) \
                        .broadcast_to([128, 2, 8])

                for j in (0, 1):
                    for k in (0, 1):
                        # theta AND theta' in one matmul: same Wph tile, pt
                        # streamed twice via a stride-0 broadcast rhs.
                        nc.tensor.matmul(pab[:, 16 * j:16 * j + 16],
                                         wtile(2 * j + k), ptb(k),
                                         start=False, stop=False)
                        # a (pre-scaled), written twice the same way
                        nc.tensor.matmul(pab[:, 32 + 16 * j:32 + 16 * j + 16],
                                         wtile(8 + 2 * j + k), ptb(k),
                                         start=False, stop=(j == 1 and k == 1))
                        # f' (pre-scaled 0.5)
                        nc.tensor.matmul(pg[:, 32 + 8 * j:32 + 8 * j + 8],
                                         wtile(12 + 2 * j + k),
                                         pt[:, 8 * k:8 * k + 8],
                                         start=False, stop=False)
                # ---- s2: sin/cos + anew shift + casa --------------------
                # sc cols: [sin-j0 cos-j0 sin-j1 cos-j1]
                nc.scalar.activation(sc[:], pab[:, 0:32], AF.Sin)
                nc.vector.scalar_tensor_tensor(casa[:], pab[:, 32:64],
                                               float(sb), sc[:],
                                               OP.add, OP.mult)
                # ---- s3: bottom contractions ----------------------------
                for j in (0, 1):
                    for k in (0, 1):
                        # gc-bot: streams ca = anew*cos (block 16k+8)
                        nc.tensor.matmul(pg[:, 8 * j:8 * j + 8],
                                         wtile(24 + 2 * j + k),
                                         casa[:, 16 * k + 8:16 * k + 16],
                                         start=False, stop=False)
                        # gs-bot: streams sa = anew*sin (block 16k)
                        nc.tensor.matmul(pg[:, 16 + 8 * j:16 + 8 * j + 8],
                                         wtile(28 + 2 * j + k),
                                         casa[:, 16 * k:16 * k + 8],
                                         start=False, stop=(j == 1 and k == 1))
                gbf = wpool.tile([128, 48], F16, tag="gbf")
                nc.scalar.activation(gbf[:], pg[:], AF.Tanh)
                # ---- s4: state update -----------------------------------
                ue.tensor_tensor(d32[:], slot[:], gbf[:, 0:32], OP.subtract)
                ue.scalar_tensor_tensor(m232[:], bcast2(gbf[:, 32:48], 16),
                                        1.0, d32[:], OP.add, OP.mult)
                ue.scalar_tensor_tensor(nslot, m232[:], 0.5, gbf[:, 0:32],
                                        OP.mult, OP.add)

            # ----- projection: I/Q = WI.hI / WQ.hQ over all t --------------
            iqs = cpool.tile([1, 2 * Tn * BL], F32, tag="iqs")
            nwin = (Tn + PW - 1) // PW
            for w in range(nwin):
                tc0 = PW * w
                tlen = min(PW, Tn - tc0)
                for q in (0, 1):
                    pp = pproj.tile([1, 8 * PW], F32, tag="pp")
                    for j in (0, 1):
                        stc = 32 * (tc0 + 1) + 16 * q + 8 * j
                        rhs = buf[:, stc:stc + 32 * tlen] \
                            .rearrange("p (t b) -> p t b", t=tlen)[:, :, 0:8]
                        nc.tensor.matmul(pp[:, 0:8 * tlen],
                                         wp[:, 2 * q + j:2 * q + j + 1],
                                         rhs, start=(j == 0), stop=(j == 1))
                    dst = iqs[0:1, q * Tn * BL + 8 * tc0:
                              q * Tn * BL + 8 * (tc0 + tlen)]
                    if (w + q) % 2 == 0:
                        nc.scalar.copy(dst, pp[:, 0:8 * tlen])
                    else:
                        nc.vector.tensor_copy(dst, pp[:, 0:8 * tlen])
            nc.sync.dma_start(out_d[0:1, 0:2 * Tn * BL], iqs[:])

    nc.compile()
    _PROG_CACHE[key] = nc
    return nc


# ---------------------------------------------------------------------------
def build_loop_program(R, U=16, sb=0.0, proj_only=False):
    """Timing-only variant: the U-step body (state ping-ponging through a
    (U+1)-slot window, x-terms re-reading one chunk) wrapped in a hardware
    For_i(0, R) loop, so one device call executes R*U steps from a tiny
    program.  Values are meaningless across iterations; the per-step WORK
    (instruction mix, widths, deps) matches build_program exactly.
    proj_only=True instead loops the projection pass over a Tn=CH window."""
    key = ("loop", R, U, float(sb), proj_only, AN_ENG, UPD_ENG, PT_ENG)
    if key in _PROG_CACHE:
        return _PROG_CACHE[key]
    _pin_tables()
    nch = 1
    nc = bacc.Bacc("TRN2", target_bir_lowering=False, debug=False,
                   num_devices=NCORES)

    w1_d = nc.dram_tensor("W1", [128, NT * 128], F16, kind="ExternalInput").ap()
    xwa_d = nc.dram_tensor("XWA", [5, 128], F16, kind="ExternalInput").ap()
    xwg_d = nc.dram_tensor("XWG", [6, 128], F16, kind="ExternalInput").ap()
    onesg_d = nc.dram_tensor("ONESG", [6, 48], F16, kind="ExternalInput").ap()
    xba_d = nc.dram_tensor("XBA", [nch, 5, CH * 64], F16,
                           kind="ExternalInput").ap()
    wp_d = nc.dram_tensor("WP", [128, 4], F16, kind="ExternalInput").ap()
    s0_d = nc.dram_tensor("S0", [128, 32], F16, kind="ExternalInput").ap()
    Tw = CH if proj_only else U
    out_d = nc.dram_tensor("OUT", [1, 2 * Tw * BL], F32,
                           kind="ExternalOutput").ap()

    with tile.TileContext(nc, trace_sim=False) as tc:
        with (
            tc.tile_pool(name="const", bufs=1) as cpool,
            tc.tile_pool(name="buf", bufs=1) as bufpool,
            tc.tile_pool(name="work", bufs=3) as wpool,
            tc.tile_pool(name="pab", bufs=2, space="PSUM") as ppab,
            tc.tile_pool(name="pg", bufs=2, space="PSUM") as ppg,
            tc.tile_pool(name="proj", bufs=2, space="PSUM") as pproj,
        ):
            wt1 = cpool.tile([128, NT * 128], F16, tag="wt1")
            xwa = cpool.tile([5, 128], F16, tag="xwa")
            xwg = cpool.tile([6, 128], F16, tag="xwg")
            onesg = cpool.tile([6, 48], F16, tag="onesg")
            wp = cpool.tile([128, 4], F16, tag="wp")
            xba = cpool.tile([5, CH * 64], F16, tag="xba")
            buf = bufpool.tile([128, 32 * (Tw + 2)], F16, tag="buf")
            iqs = cpool.tile([1, 2 * Tw * BL], F32, tag="iqs")

            nc.sync.dma_start(wt1[:], w1_d)
            nc.sync.dma_start(xwa[:], xwa_d)
            nc.sync.dma_start(xwg[:], xwg_d)
            nc.sync.dma_start(onesg[:], onesg_d)
            nc.sync.dma_start(wp[:], wp_d)
            nc.sync.dma_start(xba[:], xba_d[0])
            nc.sync.dma_start(buf[:, 0:32], s0_d)

            def wtile(i):
                return wt1[:, 128 * i:128 * (i + 1)]

            ane = {"v": nc.vector, "g": nc.gpsimd}[AN_ENG]
            ue = {"v": nc.vector, "g": nc.gpsimd}[UPD_ENG]
            pte = {"v": nc.vector, "g": nc.gpsimd}[PT_ENG]

            def step(t):
                s = t % CH
                slot = buf[:, 32 * t:32 * t + 32]
                nslot = buf[:, 32 * (t + 1):32 * (t + 1) + 32]
                pab = ppab.tile([128, 64], F32, tag="pab")
                pg = ppg.tile([128, 48], F32, tag="pg")
                pt = wpool.tile([128, 16], F16, tag="pt")
                sc = wpool.tile([128, 32], F16, tag="sc")
                casa = wpool.tile([128, 32], F16, tag="casa")
                d32 = wpool.tile([128, 32], F16, tag="d32")
                m232 = wpool.tile([128, 32], F16, tag="m232")

                nc.tensor.matmul(pg[:], xwg[:], onesg[:], start=True,
                                 stop=False)
                for j in (0, 1):
                    for k in (0, 1):
                        nc.tensor.matmul(pg[:, 8 * j:8 * j + 8],
                                         wtile(16 + 2 * j + k),
                                         slot[:, 8 * k:8 * k + 8],
                                         start=False, stop=False)
                        nc.tensor.matmul(pg[:, 16 + 8 * j:16 + 8 * j + 8],
                                         wtile(20 + 2 * j + k),
                                         slot[:, 16 + 8 * k:16 + 8 * k + 8],
                                         start=False, stop=False)
                pte.tensor_mul(pt[:], slot[:, 0:16], slot[:, 16:32])
                nc.tensor.matmul(pab[:], xwa[:],
                                 xba[:, 64 * s:64 * s + 64],
                                 start=True, stop=False)

                def ptb(k):
                    return pt[:, 8 * k:8 * k + 8] \
                        .rearrange("p (o f) -> p o f", o=1) \
                        .broadcast_to([128, 2, 8])

                for j in (0, 1):
                    for k in (0, 1):
                        nc.tensor.matmul(pab[:, 16 * j:16 * j + 16],
                                         wtile(2 * j + k), ptb(k),
                                         start=False, stop=False)
                        nc.tensor.matmul(pab[:, 32 + 16 * j:32 + 16 * j + 16],
                                         wtile(8 + 2 * j + k), ptb(k),
                                         start=False, stop=(j == 1 and k == 1))
                        nc.tensor.matmul(pg[:, 32 + 8 * j:32 + 8 * j + 8],
                                         wtile(12 + 2 * j + k),
                                         pt[:, 8 * k:8 * k + 8],
                                         start=False, stop=False)
                nc.scalar.activation(sc[:], pab[:, 0:32], AF.Sin)
                nc.vector.scalar_tensor_tensor(casa[:], pab[:, 32:64],
                                               float(sb), sc[:],
                                               OP.add, OP.mult)
                for j in (0, 1):
                    for k in (0, 1):
                        nc.tensor.matmul(pg[:, 8 * j:8 * j + 8],
                                         wtile(24 + 2 * j + k),
                                         casa[:, 16 * k + 8:16 * k + 16],
                                         start=False, stop=False)
                        nc.tensor.matmul(pg[:, 16 + 8 * j:16 + 8 * j + 8],
                                         wtile(28 + 2 * j + k),
                                         casa[:, 16 * k:16 * k + 8],
                                         start=False, stop=(j == 1 and k == 1))
                gbf = wpool.tile([128, 48], F16, tag="gbf")
                nc.scalar.activation(gbf[:], pg[:], AF.Tanh)
                ue.tensor_tensor(d32[:], slot[:], gbf[:, 0:32], OP.subtract)
                ue.scalar_tensor_tensor(m232[:], bcast2(gbf[:, 32:48], 16),
                                        1.0, d32[:], OP.add, OP.mult)
                ue.scalar_tensor_tensor(nslot, m232[:], 0.5, gbf[:, 0:32],
                                        OP.mult, OP.add)

            def bcast2(ap, w):
                return ap.rearrange("p (o f) -> p o f", o=1) \
                    .broadcast_to([128, 2, w])

            def proj_pass():
                nwin = (Tw + PW - 1) // PW
                for w in range(nwin):
                    tc0 = PW * w
                    tlen = min(PW, Tw - tc0)
                    for q in (0, 1):
                        pp = pproj.tile([1, 8 * PW], F32, tag="pp")
                        for j in (0, 1):
                            stc = 32 * (tc0 + 1) + 16 * q + 8 * j
                            rhs = buf[:, stc:stc + 32 * tlen] \
                                .rearrange("p (t b) -> p t b", t=tlen)[:, :, 0:8]
                            nc.tensor.matmul(pp[:, 0:8 * tlen],
                                             wp[:, 2 * q + j:2 * q + j + 1],
                                             rhs, start=(j == 0), stop=(j == 1))
                        dst = iqs[0:1, q * Tw * BL + 8 * tc0:
                                  q * Tw * BL + 8 * (tc0 + tlen)]
                        if (w + q) % 2 == 0:
                            nc.scalar.copy(dst, pp[:, 0:8 * tlen])
                        else:
                            nc.vector.tensor_copy(dst, pp[:, 0:8 * tlen])

            with tc.For_i(0, R):
                if proj_only:
                    proj_pass()
                else:
                    for t in range(U):
                        step(t)
            if proj_only:
                nc.sync.dma_start(out_d[0:1, :], iqs[:])
            else:
                nc.vector.tensor_copy(iqs[0:1, 0:32], buf[0:1, 0:32])
                nc.sync.dma_start(out_d[0:1, 0:32], iqs[0:1, 0:32])

    nc.compile()
    _PROG_CACHE[key] = nc
    return nc


# ---------------------------------------------------------------------------
def prepare_inputs(inputs, Tn=T):
    """Host-side preprocessing: weight packing + per-core input maps."""
    f16 = np.float16
    x = np.asarray(inputs["x"], np.float32)
    hI0 = np.asarray(inputs["hI_0"], np.float32)[0]
    hQ0 = np.asarray(inputs["hQ_0"], np.float32)[0]
    c1 = float(np.asarray(inputs["c1"])[0])
    c2 = float(np.asarray(inputs["c2"])[0])
    c3 = float(np.asarray(inputs["c3"])[0])
    sc = c1 + c2 + c3
    sb = -(c1 / 3.0 + 2.0 * c2 / 3.0 + c3)
    Wa = np.asarray(inputs["Wa"], np.float32)[0]
    Wah = np.asarray(inputs["Wah"], np.float32)
    Wp1 = np.asarray(inputs["Wp1"], np.float32)[0]
    Wph = np.asarray(inputs["Wph"], np.float32)
    Wf = np.asarray(inputs["Wf"], np.float32)
    bf = np.asarray(inputs["bf"], np.float32)
    Wgc = np.asarray(inputs["Wgc"], np.float32)
    bgc = np.asarray(inputs["bgc"], np.float32)
    Wgs = np.asarray(inputs["Wgs"], np.float32)
    bgs = np.asarray(inputs["bgs"], np.float32)
    WI = np.asarray(inputs["WI"], np.float32)
    WQ = np.asarray(inputs["WQ"], np.float32)

    def tiles4(W):
        return [W[128 * k:128 * (k + 1), 128 * j:128 * (j + 1)]
                for j in (0, 1) for k in (0, 1)]

    tl = []
    tl += tiles4(Wph)               # 0-3 theta
    tl += tiles4(Wph)               # 4-7 theta'
    tl += tiles4(sc * Wah)          # 8-11 a (pre-scaled)
    tl += tiles4(0.5 * Wf)          # 12-15 f'
    tl += tiles4(Wgc[:H])           # 16-19 gc top
    tl += tiles4(Wgs[:H])           # 20-23 gs top
    tl += tiles4(Wgc[H:])           # 24-27 gc bot (ca)
    tl += tiles4(Wgs[H:])           # 28-31 gs bot (sa)
    W1 = np.concatenate(tl, axis=1).astype(f16)

    XWA = np.stack([Wp1[0:128], Wp1[128:256], np.ones(128, np.float32),
                    (sc * Wa)[0:128], (sc * Wa)[128:256]]).astype(f16)
    XWG = np.stack([bgc[0:128], bgc[128:256], bgs[0:128], bgs[128:256],
                    0.5 * bf[0:128], 0.5 * bf[128:256]]).astype(f16)
    ONESG = np.zeros((6, 48), np.float32)
    for i in range(6):
        ONESG[i, 8 * i:8 * i + 8] = 1.0
    ONESG = ONESG.astype(f16)
    WP = np.stack([WI[0:128], WI[128:256], WQ[0:128], WQ[128:256]],
                  axis=1).astype(f16)

    nch = max(1, (Tn + CH - 1) // CH)
    in_maps = []
    for c in range(NCORES):
        bs = slice(BL * c, BL * (c + 1))
        x1p = np.zeros((nch, CH, BL), np.float32)
        x0p = np.zeros((nch, CH, BL), np.float32)
        x1p.reshape(-1, BL)[:Tn] = x[bs, :Tn, 1].T
        x0p.reshape(-1, BL)[:Tn] = x[bs, :Tn, 0].T
        XBA = np.zeros((nch, 5, CH, 64), np.float32)
        XBA[:, 0, :, 0:8] = x1p      # theta-j0
        XBA[:, 0, :, 8:16] = x1p     # theta'-j0
        XBA[:, 1, :, 16:24] = x1p    # theta-j1
        XBA[:, 1, :, 24:32] = x1p    # theta'-j1
        XBA[:, 2, :, 8:16] = np.pi / 2
        XBA[:, 2, :, 24:32] = np.pi / 2
        XBA[:, 3, :, 32:48] = np.repeat(x0p, 2, axis=0).reshape(nch, 2, CH, 8).transpose(0, 2, 1, 3).reshape(nch, CH, 16)
        XBA[:, 4, :, 48:64] = XBA[:, 3, :, 32:48]
        S0 = np.zeros((128, 32), np.float32)
        for j in (0, 1):
            S0[:, 8 * j:8 * j + 8] = hI0[bs, 128 * j:128 * (j + 1)].T
            S0[:, 16 + 8 * j:24 + 8 * j] = hQ0[bs, 128 * j:128 * (j + 1)].T
        in_maps.append({
            "W1": W1, "XWA": XWA, "XWG": XWG, "ONESG": ONESG,
            "WP": WP, "S0": S0.astype(f16),
            "XBA": XBA.reshape(nch, 5, CH * 64).astype(f16),
        })
    return in_maps, sb


def assemble(results, inputs, Tn=T):
    bI = float(np.asarray(inputs["bI"])[0])
    bQ = float(np.asarray(inputs["bQ"])[0])
    out = np.zeros((B, Tn, 2), np.float32)
    for c in range(NCORES):
        arr = results[c]["OUT"].reshape(-1)[:2 * Tn * BL]
        for q in (0, 1):
            seg = arr[q * Tn * BL:(q + 1) * Tn * BL].reshape(Tn, BL)
            rows = slice(BL * c, BL * (c + 1))
            out[rows, :, q] = seg.T + (bI if q == 0 else bQ)
    return out


def kernel(**inputs) -> np.ndarray:
    in_maps, sb = prepare_inputs(inputs, T)
    nc = build_program(T, sb)
    res = run_bass_kernel_spmd(nc, in_maps, list(range(NCORES)))
    return assemble(res.results, inputs, T)
